# revision 1
# baseline (speedup 1.0000x reference)
"""GCNII-style GNN message-passing layer on 8 Trainium2 NeuronCores.

Strategy (dst-sharded graph parallelism, x replicated):
  - 100k nodes split into 8 contiguous dst ranges (12500 per core).
  - Each core owns the segment-sum + post-processing for its dst range.
  - Edges are grouped by (dst supertile, src piece, dst tile) on the host;
    chunk = 128 edge slots, pure in (dst tile, src piece).
  - Source features are fetched with the SWDGE dma_gather instruction
    (int16 idx relative to a <=25000-row piece window of x, idx replicated
    across the 8 Q7 cores); one gather op per (supertile, piece).
  - The segment-sum scatter runs on the TensorEngine: per chunk,
    psum_hT[tile] += matmul(lhsT=msgs[e,f], rhs=S[e,d]) where S is a
    host-built one-hot scatter block weighted by (1-alpha)*edge_weight.
  - Post-processing per 128-dst tile is done feature-major ([f, d]):
    convex-combine with 0.1*init_x^T, beta-weighted linear via a single
    matmul with (0.5*W_lin + 0.5*I)^T, LayerNorm over the feature dim via
    ones-vector column-stat matmuls + rank-1 broadcast matmuls,
    relu(gamma*y + beta) fused on the Scalar engine, residual W_res @ x^T
    accumulated in PSUM, then out^T streamed to DRAM.
  - Host reassembles/transposes the 8 out^T shards into [100000, 128] f32.
"""
import sys

for _p in ("/opt/trn_rl_repo",):
    if _p not in sys.path:
        sys.path.insert(0, _p)

import numpy as np
import ml_dtypes

import concourse.bacc as bacc
import concourse.bass as bass
import concourse.mybir as mybir
from concourse.tile import TileContext
from concourse.bass_utils import run_bass_kernel_spmd

P = 128
LN_EPS = 1e-5
ALPHA = 0.1
BETA = 0.5
N_CORES = 8
PIECE_W = 25000
ST = 4           # dst tiles per supertile (gather batching)


def _host_prep(edge_index, edge_weight, x, init_x, W_lin, b_lin, W_res, b_res,
               ln_gamma, ln_beta, msgs_f32=False):
    N, D = x.shape
    assert D == P
    n_cores = N_CORES
    shard = -(-N // n_cores)
    n_tiles = -(-shard // P)
    shard_pad = n_tiles * P
    piece_w = min(PIECE_W, N)
    n_pieces = -(-N // piece_w)
    st = ST
    n_st = -(-n_tiles // st)
    G = n_st * n_pieces * st

    dst = np.asarray(edge_index[0], dtype=np.int64)
    src = np.asarray(edge_index[1], dtype=np.int64)
    w = np.asarray(edge_weight, dtype=np.float32) * (1.0 - ALPHA)

    core = dst // shard
    tile = (dst % shard) // P
    si = tile // st
    tin = tile % st
    piece = src // piece_w
    g = (si * n_pieces + piece) * st + tin
    gkey = core * G + g
    order = np.argsort(gkey, kind='stable')
    src_s = src[order]
    w_s = w[order]
    j_s = (dst[order] % shard) % P
    gkey_s = gkey[order]

    counts = np.bincount(gkey_s, minlength=n_cores * G).reshape(n_cores, G)
    gg = np.arange(G)
    g_tile = (gg // (n_pieces * st)) * st + gg % st
    exists = g_tile < n_tiles
    n_g = np.where(exists, np.maximum(1, -(-counts.max(axis=0) // P)), 0)
    base = np.concatenate([[0], np.cumsum(n_g)]).astype(np.int64)
    T = int(base[-1])

    g_s = gkey_s % G
    start_of_key = np.concatenate(
        [[0], np.cumsum(np.bincount(gkey_s, minlength=n_cores * G))])
    rank = np.arange(len(gkey_s)) - start_of_key[gkey_s]
    slot = base[g_s] * P + rank

    gdt = np.float32 if msgs_f32 else ml_dtypes.bfloat16
    x_g = np.ascontiguousarray(x.astype(gdt))

    Mw = (BETA * np.asarray(W_lin, np.float32)
          + (1.0 - BETA) * np.eye(P, dtype=np.float32))
    MwT = np.ascontiguousarray(Mw.T.astype(np.float32))
    WresT = np.ascontiguousarray(np.asarray(W_res, np.float32).T)
    bhalf = (BETA * np.asarray(b_lin, np.float32)).reshape(1, P)
    bres = np.asarray(b_res, np.float32).reshape(1, P)
    gamma = np.asarray(ln_gamma, np.float32).reshape(P, 1)
    beta = np.asarray(ln_beta, np.float32).reshape(P, 1)

    core_s = gkey_s // G
    piece_s = (gkey_s % G) // st % n_pieces
    in_maps = []
    for c in range(n_cores):
        sel = core_s == c
        sl = slot[sel]
        idx_flat = np.zeros(T * P, np.int32)
        idx_flat[sl] = (src_s[sel] - piece_s[sel] * piece_w).astype(np.int32)
        idx16 = np.zeros((16, T * P // 16), np.int16)
        ii = np.arange(T * P)
        idx16[ii % 16, ii // 16] = idx_flat.astype(np.int16)
        idx16 = np.tile(idx16, (8, 1))

        S_arr = np.zeros((T, P, P), np.float32)
        S_arr[sl // P, sl % P, j_s[sel]] = w_s[sel]

        lo, hi = c * shard, min((c + 1) * shard, N)
        xT_c = np.zeros((P, shard_pad), np.float32)
        xT_c[:, :hi - lo] = x[lo:hi].T.astype(np.float32)
        ixT_c = np.zeros((P, shard_pad), np.float32)
        ixT_c[:, :hi - lo] = (ALPHA * np.asarray(init_x[lo:hi], np.float32)).T

        in_maps.append({
            "xg": x_g,
            "idx": np.ascontiguousarray(idx16),
            "S": np.ascontiguousarray(
                S_arr.transpose(1, 0, 2).reshape(P, T * P).astype(ml_dtypes.bfloat16)),
            "ixT": np.ascontiguousarray(ixT_c.astype(ml_dtypes.bfloat16)),
            "xT": xT_c,
            "MwT": MwT, "WresT": WresT, "bhalf": bhalf, "bres": bres,
            "gamma": gamma, "beta": beta,
        })
    meta = dict(N=N, n_cores=n_cores, n_tiles=n_tiles, n_pieces=n_pieces,
                piece_w=piece_w, st=st, n_st=n_st, shard=shard,
                shard_pad=shard_pad, T=T, n_g=n_g.tolist(), msgs_f32=msgs_f32)
    return in_maps, meta


def _build(meta, msgs_bufs=8, s_bufs=3, repeat=1):
    N = meta["N"]; n_tiles = meta["n_tiles"]; n_pieces = meta["n_pieces"]
    piece_w = meta["piece_w"]; st = meta["st"]; n_st = meta["n_st"]
    shard_pad = meta["shard_pad"]; T = meta["T"]
    n_g = np.asarray(meta["n_g"]); n_cores = meta["n_cores"]
    gdt = mybir.dt.float32 if meta["msgs_f32"] else mybir.dt.bfloat16
    f32 = mybir.dt.float32
    bf16 = mybir.dt.bfloat16
    base = np.concatenate([[0], np.cumsum(n_g)]).astype(int)

    def key(si, q, tin):
        return (si * n_pieces + q) * st + tin

    nc = bacc.Bacc("TRN2", target_bir_lowering=False, debug=False,
                   num_devices=n_cores)
    xg = nc.dram_tensor("xg", [N, P], gdt, kind="ExternalInput")
    idx = nc.dram_tensor("idx", [P, T * P // 16], mybir.dt.int16, kind="ExternalInput")
    S = nc.dram_tensor("S", [P, T * P], bf16, kind="ExternalInput")
    ixT = nc.dram_tensor("ixT", [P, shard_pad], bf16, kind="ExternalInput")
    xT = nc.dram_tensor("xT", [P, shard_pad], f32, kind="ExternalInput")
    MwT = nc.dram_tensor("MwT", [P, P], f32, kind="ExternalInput")
    WresT = nc.dram_tensor("WresT", [P, P], f32, kind="ExternalInput")
    bhalf = nc.dram_tensor("bhalf", [1, P], f32, kind="ExternalInput")
    bres = nc.dram_tensor("bres", [1, P], f32, kind="ExternalInput")
    gamma = nc.dram_tensor("gamma", [P, 1], f32, kind="ExternalInput")
    beta = nc.dram_tensor("beta", [P, 1], f32, kind="ExternalInput")
    outT = nc.dram_tensor("outT", [P, shard_pad], f32, kind="ExternalOutput")

    AF = mybir.ActivationFunctionType
    ALU = mybir.AluOpType

    with TileContext(nc) as tc:
        with tc.tile_pool(name="const", bufs=1) as cpool, \
             tc.tile_pool(name="msgs", bufs=msgs_bufs) as mpool, \
             tc.tile_pool(name="sblk", bufs=s_bufs) as spool, \
             tc.tile_pool(name="stream", bufs=3) as stpool, \
             tc.tile_pool(name="post", bufs=2) as ppool, \
             tc.tile_pool(name="psumh", bufs=2, space="PSUM") as php, \
             tc.tile_pool(name="psumo", bufs=1, space="PSUM") as pop:

            idx_sb = cpool.tile([P, T * P // 16], mybir.dt.int16, name="idx_sb")
            nc.sync.dma_start(out=idx_sb[:], in_=idx[:, :])
            MwT_sb = cpool.tile([P, P], f32, name="MwT_sb")
            nc.sync.dma_start(out=MwT_sb[:], in_=MwT[:, :])
            WresT_sb = cpool.tile([P, P], f32, name="WresT_sb")
            nc.sync.dma_start(out=WresT_sb[:], in_=WresT[:, :])
            bhalf_sb = cpool.tile([1, P], f32, name="bhalf_sb")
            nc.sync.dma_start(out=bhalf_sb[:], in_=bhalf[:, :])
            bres_sb = cpool.tile([1, P], f32, name="bres_sb")
            nc.sync.dma_start(out=bres_sb[:], in_=bres[:, :])
            gamma_sb = cpool.tile([P, 1], f32, name="gamma_sb")
            nc.sync.dma_start(out=gamma_sb[:], in_=gamma[:, :])
            beta_sb = cpool.tile([P, 1], f32, name="beta_sb")
            nc.sync.dma_start(out=beta_sb[:], in_=beta[:, :])
            ones_row = cpool.tile([1, P], f32, name="ones_row")
            nc.vector.memset(ones_row[:], 1.0)
            colmean = cpool.tile([P, 1], f32, name="colmean")
            nc.vector.memset(colmean[:], 1.0 / P)
            eps_sb = cpool.tile([1, 1], f32, name="eps_sb")
            nc.vector.memset(eps_sb[:], LN_EPS)

            def body(_iv=None):
                for si in range(n_st):
                    t0, t1 = si * st, min((si + 1) * st, n_tiles)
                    mts = {}
                    for q in range(n_pieces):
                        c0 = int(base[key(si, q, 0)])
                        c1 = int(base[key(si, q, t1 - t0 - 1) + 1])
                        nck = c1 - c0
                        npi = nck * P
                        mt = mpool.tile([P, nck * P], gdt, name=f"mt{si}_{q}", tag="mt")
                        pbase = q * piece_w
                        phi = min(N - pbase, 32768)
                        nc.gpsimd.dma_gather(
                            out_ap=mt[:].rearrange("p (c d) -> p c d", d=P),
                            in_ap=xg[pbase:pbase + phi, :],
                            idxs_ap=idx_sb[:, c0 * P // 16:c1 * P // 16],
                            num_idxs=npi,
                            num_idxs_reg=npi,
                            elem_size=P,
                            single_packet=False,
                        )
                        mts[q] = (mt, c0)
                    for tin in range(t1 - t0):
                        t = t0 + tin
                        psum_h = php.tile([P, P], f32, name=f"ph{t}", tag="ph", space="PSUM")
                        tot = sum(int(base[key(si, q, tin) + 1] - base[key(si, q, tin)])
                                  for q in range(n_pieces))
                        done = 0
                        for q in range(n_pieces):
                            gk = key(si, q, tin)
                            c0, c1 = int(base[gk]), int(base[gk + 1])
                            stile = spool.tile([P, (c1 - c0) * P], bf16,
                                               name=f"st{gk}", tag="st")
                            nc.sync.dma_start(out=stile[:], in_=S[:, c0 * P:c1 * P])
                            mt, mc0 = mts[q]
                            for ci in range(c0, c1):
                                done += 1
                                nc.tensor.matmul(
                                    out=psum_h[:],
                                    lhsT=mt[:, (ci - mc0) * P:(ci - mc0 + 1) * P],
                                    rhs=stile[:, (ci - c0) * P:(ci - c0 + 1) * P],
                                    start=(done == 1), stop=(done == tot),
                                )

                        tsl = slice(t * P, (t + 1) * P)
                        ixt_t = stpool.tile([P, P], bf16, name=f"ix{t}", tag="ixt")
                        nc.sync.dma_start(out=ixt_t[:], in_=ixT[:, tsl])
                        xt_t = stpool.tile([P, P], f32, name=f"xt{t}", tag="xt")
                        nc.sync.dma_start(out=xt_t[:], in_=xT[:, tsl])

                        h_sb = ppool.tile([P, P], f32, name=f"h{t}", tag="h")
                        nc.vector.tensor_tensor(out=h_sb[:], in0=psum_h[:],
                                                in1=ixt_t[:], op=ALU.add)
                        psum_u = pop.tile([P, P], f32, name=f"pu{t}", tag="pu",
                                          space="PSUM")
                        nc.tensor.matmul(out=psum_u[:], lhsT=MwT_sb[:], rhs=h_sb[:],
                                         start=True, stop=False)
                        nc.tensor.matmul(out=psum_u[:], lhsT=bhalf_sb[:],
                                         rhs=ones_row[:], start=False, stop=True)
                        u_sb = ppool.tile([P, P], f32, name=f"u{t}", tag="u")
                        nc.scalar.activation(out=u_sb[:], in_=psum_u[:], func=AF.Copy)
                        usq_sb = ppool.tile([P, P], f32, name=f"usq{t}", tag="usq")
                        nc.scalar.activation(out=usq_sb[:], in_=psum_u[:],
                                             func=AF.Square)

                        psum_st = pop.tile([1, 2 * P], f32, name=f"pst{t}", tag="pst",
                                           space="PSUM")
                        nc.tensor.matmul(out=psum_st[:, 0:P], lhsT=colmean[:],
                                         rhs=u_sb[:], start=True, stop=True)
                        nc.tensor.matmul(out=psum_st[:, P:2 * P], lhsT=colmean[:],
                                         rhs=usq_sb[:], start=True, stop=True)
                        mu_sb = ppool.tile([1, P], f32, name=f"mu{t}", tag="mu")
                        nc.vector.tensor_copy(out=mu_sb[:], in_=psum_st[:, 0:P])
                        var_sb = ppool.tile([1, P], f32, name=f"var{t}", tag="var")
                        nc.vector.tensor_tensor(out=var_sb[:], in0=mu_sb[:],
                                                in1=mu_sb[:], op=ALU.mult)
                        nc.vector.tensor_tensor(out=var_sb[:], in0=psum_st[:, P:2 * P],
                                                in1=var_sb[:], op=ALU.subtract)
                        sd_sb = ppool.tile([1, P], f32, name=f"sd{t}", tag="sd")
                        nc.scalar.activation(out=sd_sb[:], in_=var_sb[:], func=AF.Sqrt,
                                             bias=eps_sb[:, 0:1])
                        rstd_sb = ppool.tile([1, P], f32, name=f"rs{t}", tag="rs")
                        nc.vector.reciprocal(out=rstd_sb[:], in_=sd_sb[:])

                        psum_b = pop.tile([P, 2 * P], f32, name=f"pb{t}", tag="pb",
                                          space="PSUM")
                        nc.tensor.matmul(out=psum_b[:, 0:P], lhsT=ones_row[:],
                                         rhs=mu_sb[:], start=True, stop=True)
                        nc.tensor.matmul(out=psum_b[:, P:2 * P], lhsT=ones_row[:],
                                         rhs=rstd_sb[:], start=True, stop=True)
                        y_sb = ppool.tile([P, P], f32, name=f"y{t}", tag="y")
                        nc.vector.tensor_tensor(out=y_sb[:], in0=u_sb[:],
                                                in1=psum_b[:, 0:P], op=ALU.subtract)
                        nc.vector.tensor_tensor(out=y_sb[:], in0=y_sb[:],
                                                in1=psum_b[:, P:2 * P], op=ALU.mult)
                        r_sb = ppool.tile([P, P], f32, name=f"r{t}", tag="r")
                        nc.scalar.activation(out=r_sb[:], in_=y_sb[:], func=AF.Relu,
                                             scale=gamma_sb[:, 0:1],
                                             bias=beta_sb[:, 0:1])

                        psum_res = pop.tile([P, P], f32, name=f"pr{t}", tag="pr",
                                            space="PSUM")
                        nc.tensor.matmul(out=psum_res[:], lhsT=WresT_sb[:], rhs=xt_t[:],
                                         start=True, stop=False)
                        nc.tensor.matmul(out=psum_res[:], lhsT=bres_sb[:],
                                         rhs=ones_row[:], start=False, stop=True)
                        o_sb = ppool.tile([P, P], f32, name=f"o{t}", tag="o")
                        nc.vector.tensor_tensor(out=o_sb[:], in0=r_sb[:],
                                                in1=psum_res[:], op=ALU.add)
                        nc.sync.dma_start(out=outT[:, tsl], in_=o_sb[:])

            for _ in range(repeat):
                body()

    nc.compile()
    return nc


def _assemble(results, meta):
    N = meta["N"]; shard = meta["shard"]; n_cores = meta["n_cores"]
    parts = []
    for c in range(n_cores):
        lo, hi = c * shard, min((c + 1) * shard, N)
        parts.append(results[c]["outT"][:, :hi - lo].T)
    return np.ascontiguousarray(np.concatenate(parts, axis=0).astype(np.float32))


def kernel(**inputs):
    in_maps, meta = _host_prep(**inputs)
    nc = _build(meta)
    res = run_bass_kernel_spmd(nc, in_maps, core_ids=list(range(N_CORES)))
    return _assemble(res.results, meta)



# revision 4
# speedup vs baseline: 2.8410x; 2.8410x over previous
"""GCNII-style GNN message-passing layer on 8 Trainium2 NeuronCores.

Strategy (dst-sharded graph parallelism, x replicated):
  - 100k nodes split into 8 contiguous dst ranges (12500 per core).
  - Each core owns the segment-sum + post-processing for its dst range.
  - Edges are grouped by (dst supertile, src piece, dst tile) on the host;
    chunk = 128 edge slots, pure in (dst tile, src piece).
  - Source features are fetched with the SWDGE dma_gather instruction
    (int16 idx relative to a <=25000-row piece window of x, idx replicated
    across the 8 Q7 cores); one gather op per (supertile, piece).
  - The segment-sum scatter runs on the TensorEngine: per chunk,
    psum_hT[tile] += matmul(lhsT=msgs[e,f], rhs=S[e,d]) where S is a
    host-built one-hot scatter block weighted by (1-alpha)*edge_weight.
  - Post-processing per 128-dst tile is done feature-major ([f, d]):
    convex-combine with 0.1*init_x^T, beta-weighted linear via a single
    matmul with (0.5*W_lin + 0.5*I)^T, LayerNorm over the feature dim via
    ones-vector column-stat matmuls + rank-1 broadcast matmuls,
    relu(gamma*y + beta) fused on the Scalar engine, residual W_res @ x^T
    accumulated in PSUM, then out^T streamed to DRAM.
  - Host reassembles/transposes the 8 out^T shards into [100000, 128] f32.
"""
import sys

for _p in ("/opt/trn_rl_repo",):
    if _p not in sys.path:
        sys.path.insert(0, _p)

import numpy as np
import ml_dtypes

import concourse.bacc as bacc
import concourse.bass as bass
import concourse.mybir as mybir
from concourse.tile import TileContext
from concourse.bass_utils import run_bass_kernel_spmd

P = 128
LN_EPS = 1e-5
ALPHA = 0.1
BETA = 0.5
N_CORES = 8
PIECE_W = 25000
ST = 4           # dst tiles per supertile (gather batching)


def _host_prep(edge_index, edge_weight, x, init_x, W_lin, b_lin, W_res, b_res,
               ln_gamma, ln_beta, msgs_f32=False):
    N, D = x.shape
    assert D == P
    n_cores = N_CORES
    shard = -(-N // n_cores)
    n_tiles = -(-shard // P)
    shard_pad = n_tiles * P
    piece_w = min(PIECE_W, N)
    n_pieces = -(-N // piece_w)
    st = ST
    n_st = -(-n_tiles // st)
    G = n_st * n_pieces * st

    dst = np.asarray(edge_index[0], dtype=np.int64)
    src = np.asarray(edge_index[1], dtype=np.int64)
    w = np.asarray(edge_weight, dtype=np.float32) * (1.0 - ALPHA)

    core = dst // shard
    tile = (dst % shard) // P
    si = tile // st
    tin = tile % st
    piece = src // piece_w
    g = (si * n_pieces + piece) * st + tin
    gkey = core * G + g
    order = np.argsort(gkey, kind='stable')
    src_s = src[order]
    w_s = w[order]
    j_s = (dst[order] % shard) % P
    gkey_s = gkey[order]

    counts = np.bincount(gkey_s, minlength=n_cores * G).reshape(n_cores, G)
    gg = np.arange(G)
    g_tile = (gg // (n_pieces * st)) * st + gg % st
    exists = g_tile < n_tiles
    n_g = np.where(exists, np.maximum(1, -(-counts.max(axis=0) // P)), 0)
    base = np.concatenate([[0], np.cumsum(n_g)]).astype(np.int64)
    T = int(base[-1])

    g_s = gkey_s % G
    start_of_key = np.concatenate(
        [[0], np.cumsum(np.bincount(gkey_s, minlength=n_cores * G))])
    rank = np.arange(len(gkey_s)) - start_of_key[gkey_s]
    slot = base[g_s] * P + rank

    gdt = np.float32 if msgs_f32 else ml_dtypes.bfloat16
    x_g = np.ascontiguousarray(x.astype(gdt))

    Mw = (BETA * np.asarray(W_lin, np.float32)
          + (1.0 - BETA) * np.eye(P, dtype=np.float32))
    MwT = np.ascontiguousarray(Mw.T.astype(np.float32))
    WresT = np.ascontiguousarray(np.asarray(W_res, np.float32).T)
    bhalf = (BETA * np.asarray(b_lin, np.float32)).reshape(1, P)
    bres = np.asarray(b_res, np.float32).reshape(1, P)
    gamma = np.asarray(ln_gamma, np.float32).reshape(P, 1)
    beta = np.asarray(ln_beta, np.float32).reshape(P, 1)

    core_s = gkey_s // G
    piece_s = (gkey_s % G) // st % n_pieces
    in_maps = []
    for c in range(n_cores):
        sel = core_s == c
        sl = slot[sel]
        idx_flat = np.zeros(T * P, np.int32)
        idx_flat[sl] = (src_s[sel] - piece_s[sel] * piece_w).astype(np.int32)
        idx16 = np.zeros((16, T * P // 16), np.int16)
        ii = np.arange(T * P)
        idx16[ii % 16, ii // 16] = idx_flat.astype(np.int16)
        idx16 = np.tile(idx16, (8, 1))

        S_arr = np.zeros((T, P, P), np.float32)
        S_arr[sl // P, sl % P, j_s[sel]] = w_s[sel]

        lo, hi = c * shard, min((c + 1) * shard, N)
        xT_c = np.zeros((P, shard_pad), np.float32)
        xT_c[:, :hi - lo] = x[lo:hi].T.astype(np.float32)
        ixT_c = np.zeros((P, shard_pad), np.float32)
        ixT_c[:, :hi - lo] = (ALPHA * np.asarray(init_x[lo:hi], np.float32)).T

        in_maps.append({
            "xg": x_g,
            "idx": np.ascontiguousarray(idx16),
            "S": np.ascontiguousarray(
                S_arr.transpose(1, 0, 2).reshape(P, T * P).astype(ml_dtypes.bfloat16)),
            "ixT": np.ascontiguousarray(ixT_c.astype(ml_dtypes.bfloat16)),
            "xT": xT_c,
            "MwT": MwT, "WresT": WresT, "bhalf": bhalf, "bres": bres,
            "gamma": gamma, "beta": beta,
        })
    meta = dict(N=N, n_cores=n_cores, n_tiles=n_tiles, n_pieces=n_pieces,
                piece_w=piece_w, st=st, n_st=n_st, shard=shard,
                shard_pad=shard_pad, T=T, n_g=n_g.tolist(), msgs_f32=msgs_f32)
    return in_maps, meta


def _build(meta, msgs_bufs=8, s_bufs=3, repeat=1):
    N = meta["N"]; n_tiles = meta["n_tiles"]; n_pieces = meta["n_pieces"]
    piece_w = meta["piece_w"]; st = meta["st"]; n_st = meta["n_st"]
    shard_pad = meta["shard_pad"]; T = meta["T"]
    n_g = np.asarray(meta["n_g"]); n_cores = meta["n_cores"]
    gdt = mybir.dt.float32 if meta["msgs_f32"] else mybir.dt.bfloat16
    f32 = mybir.dt.float32
    bf16 = mybir.dt.bfloat16
    base = np.concatenate([[0], np.cumsum(n_g)]).astype(int)

    def key(si, q, tin):
        return (si * n_pieces + q) * st + tin

    nc = bacc.Bacc("TRN2", target_bir_lowering=False, debug=False,
                   num_devices=n_cores, num_swdge_queues=4)
    xg = nc.dram_tensor("xg", [N, P], gdt, kind="ExternalInput")
    idx = nc.dram_tensor("idx", [P, T * P // 16], mybir.dt.int16, kind="ExternalInput")
    S = nc.dram_tensor("S", [P, T * P], bf16, kind="ExternalInput")
    ixT = nc.dram_tensor("ixT", [P, shard_pad], bf16, kind="ExternalInput")
    xT = nc.dram_tensor("xT", [P, shard_pad], f32, kind="ExternalInput")
    MwT = nc.dram_tensor("MwT", [P, P], f32, kind="ExternalInput")
    WresT = nc.dram_tensor("WresT", [P, P], f32, kind="ExternalInput")
    bhalf = nc.dram_tensor("bhalf", [1, P], f32, kind="ExternalInput")
    bres = nc.dram_tensor("bres", [1, P], f32, kind="ExternalInput")
    gamma = nc.dram_tensor("gamma", [P, 1], f32, kind="ExternalInput")
    beta = nc.dram_tensor("beta", [P, 1], f32, kind="ExternalInput")
    outT = nc.dram_tensor("outT", [P, shard_pad], f32, kind="ExternalOutput")

    AF = mybir.ActivationFunctionType
    ALU = mybir.AluOpType

    with TileContext(nc) as tc:
        with tc.tile_pool(name="const", bufs=1) as cpool, \
             tc.tile_pool(name="msgs", bufs=msgs_bufs) as mpool, \
             tc.tile_pool(name="sblk", bufs=s_bufs) as spool, \
             tc.tile_pool(name="stream", bufs=3) as stpool, \
             tc.tile_pool(name="post", bufs=2) as ppool, \
             tc.tile_pool(name="psumh", bufs=2, space="PSUM") as php, \
             tc.tile_pool(name="psumo", bufs=1, space="PSUM") as pop:

            idx_sb = cpool.tile([P, T * P // 16], mybir.dt.int16, name="idx_sb")
            nc.sync.dma_start(out=idx_sb[:], in_=idx[:, :])
            MwT_sb = cpool.tile([P, P], f32, name="MwT_sb")
            nc.sync.dma_start(out=MwT_sb[:], in_=MwT[:, :])
            WresT_sb = cpool.tile([P, P], f32, name="WresT_sb")
            nc.sync.dma_start(out=WresT_sb[:], in_=WresT[:, :])
            bhalf_sb = cpool.tile([1, P], f32, name="bhalf_sb")
            nc.sync.dma_start(out=bhalf_sb[:], in_=bhalf[:, :])
            bres_sb = cpool.tile([1, P], f32, name="bres_sb")
            nc.sync.dma_start(out=bres_sb[:], in_=bres[:, :])
            gamma_sb = cpool.tile([P, 1], f32, name="gamma_sb")
            nc.sync.dma_start(out=gamma_sb[:], in_=gamma[:, :])
            beta_sb = cpool.tile([P, 1], f32, name="beta_sb")
            nc.sync.dma_start(out=beta_sb[:], in_=beta[:, :])
            ones_row = cpool.tile([1, P], f32, name="ones_row")
            nc.vector.memset(ones_row[:], 1.0)
            colmean = cpool.tile([P, 1], f32, name="colmean")
            nc.vector.memset(colmean[:], 1.0 / P)
            eps_sb = cpool.tile([1, 1], f32, name="eps_sb")
            nc.vector.memset(eps_sb[:], LN_EPS)

            def body(_iv=None):
                gq = [0]
                for si in range(n_st):
                    t0, t1 = si * st, min((si + 1) * st, n_tiles)
                    mts = {}
                    for q in range(n_pieces):
                        c0 = int(base[key(si, q, 0)])
                        c1 = int(base[key(si, q, t1 - t0 - 1) + 1])
                        nck = c1 - c0
                        npi = nck * P
                        mt = mpool.tile([P, nck * P], gdt, name=f"mt{si}_{q}", tag="mt")
                        pbase = q * piece_w
                        phi = min(N - pbase, 32768)
                        nc.gpsimd.dma_gather(
                            out_ap=mt[:].rearrange("p (c d) -> p c d", d=P),
                            in_ap=xg[pbase:pbase + phi, :],
                            idxs_ap=idx_sb[:, c0 * P // 16:c1 * P // 16],
                            num_idxs=npi,
                            num_idxs_reg=npi,
                            elem_size=P,
                            single_packet=False,
                            queue_num=gq[0] % 4,
                        )
                        gq[0] += 1
                        mts[q] = (mt, c0)
                    for tin in range(t1 - t0):
                        t = t0 + tin
                        psum_h = php.tile([P, P], f32, name=f"ph{t}", tag="ph", space="PSUM")
                        tot = sum(int(base[key(si, q, tin) + 1] - base[key(si, q, tin)])
                                  for q in range(n_pieces))
                        done = 0
                        for q in range(n_pieces):
                            gk = key(si, q, tin)
                            c0, c1 = int(base[gk]), int(base[gk + 1])
                            stile = spool.tile([P, (c1 - c0) * P], bf16,
                                               name=f"st{gk}", tag="st")
                            nc.sync.dma_start(out=stile[:], in_=S[:, c0 * P:c1 * P])
                            mt, mc0 = mts[q]
                            for ci in range(c0, c1):
                                done += 1
                                nc.tensor.matmul(
                                    out=psum_h[:],
                                    lhsT=mt[:, (ci - mc0) * P:(ci - mc0 + 1) * P],
                                    rhs=stile[:, (ci - c0) * P:(ci - c0 + 1) * P],
                                    start=(done == 1), stop=(done == tot),
                                )

                        tsl = slice(t * P, (t + 1) * P)
                        ixt_t = stpool.tile([P, P], bf16, name=f"ix{t}", tag="ixt")
                        nc.sync.dma_start(out=ixt_t[:], in_=ixT[:, tsl])
                        xt_t = stpool.tile([P, P], f32, name=f"xt{t}", tag="xt")
                        nc.sync.dma_start(out=xt_t[:], in_=xT[:, tsl])

                        h_sb = ppool.tile([P, P], f32, name=f"h{t}", tag="h")
                        nc.vector.tensor_tensor(out=h_sb[:], in0=psum_h[:],
                                                in1=ixt_t[:], op=ALU.add)
                        psum_u = pop.tile([P, P], f32, name=f"pu{t}", tag="pu",
                                          space="PSUM")
                        nc.tensor.matmul(out=psum_u[:], lhsT=MwT_sb[:], rhs=h_sb[:],
                                         start=True, stop=False)
                        nc.tensor.matmul(out=psum_u[:], lhsT=bhalf_sb[:],
                                         rhs=ones_row[:], start=False, stop=True)
                        u_sb = ppool.tile([P, P], f32, name=f"u{t}", tag="u")
                        nc.scalar.activation(out=u_sb[:], in_=psum_u[:], func=AF.Copy)
                        usq_sb = ppool.tile([P, P], f32, name=f"usq{t}", tag="usq")
                        nc.scalar.activation(out=usq_sb[:], in_=psum_u[:],
                                             func=AF.Square)

                        psum_st = pop.tile([1, 2 * P], f32, name=f"pst{t}", tag="pst",
                                           space="PSUM")
                        nc.tensor.matmul(out=psum_st[:, 0:P], lhsT=colmean[:],
                                         rhs=u_sb[:], start=True, stop=True)
                        nc.tensor.matmul(out=psum_st[:, P:2 * P], lhsT=colmean[:],
                                         rhs=usq_sb[:], start=True, stop=True)
                        mu_sb = ppool.tile([1, P], f32, name=f"mu{t}", tag="mu")
                        nc.vector.tensor_copy(out=mu_sb[:], in_=psum_st[:, 0:P])
                        var_sb = ppool.tile([1, P], f32, name=f"var{t}", tag="var")
                        nc.vector.tensor_tensor(out=var_sb[:], in0=mu_sb[:],
                                                in1=mu_sb[:], op=ALU.mult)
                        nc.vector.tensor_tensor(out=var_sb[:], in0=psum_st[:, P:2 * P],
                                                in1=var_sb[:], op=ALU.subtract)
                        sd_sb = ppool.tile([1, P], f32, name=f"sd{t}", tag="sd")
                        nc.scalar.activation(out=sd_sb[:], in_=var_sb[:], func=AF.Sqrt,
                                             bias=eps_sb[:, 0:1])
                        rstd_sb = ppool.tile([1, P], f32, name=f"rs{t}", tag="rs")
                        nc.vector.reciprocal(out=rstd_sb[:], in_=sd_sb[:])

                        psum_b = pop.tile([P, 2 * P], f32, name=f"pb{t}", tag="pb",
                                          space="PSUM")
                        nc.tensor.matmul(out=psum_b[:, 0:P], lhsT=ones_row[:],
                                         rhs=mu_sb[:], start=True, stop=True)
                        nc.tensor.matmul(out=psum_b[:, P:2 * P], lhsT=ones_row[:],
                                         rhs=rstd_sb[:], start=True, stop=True)
                        y_sb = ppool.tile([P, P], f32, name=f"y{t}", tag="y")
                        nc.vector.tensor_tensor(out=y_sb[:], in0=u_sb[:],
                                                in1=psum_b[:, 0:P], op=ALU.subtract)
                        nc.vector.tensor_tensor(out=y_sb[:], in0=y_sb[:],
                                                in1=psum_b[:, P:2 * P], op=ALU.mult)
                        r_sb = ppool.tile([P, P], f32, name=f"r{t}", tag="r")
                        nc.scalar.activation(out=r_sb[:], in_=y_sb[:], func=AF.Relu,
                                             scale=gamma_sb[:, 0:1],
                                             bias=beta_sb[:, 0:1])

                        psum_res = pop.tile([P, P], f32, name=f"pr{t}", tag="pr",
                                            space="PSUM")
                        nc.tensor.matmul(out=psum_res[:], lhsT=WresT_sb[:], rhs=xt_t[:],
                                         start=True, stop=False)
                        nc.tensor.matmul(out=psum_res[:], lhsT=bres_sb[:],
                                         rhs=ones_row[:], start=False, stop=True)
                        o_sb = ppool.tile([P, P], f32, name=f"o{t}", tag="o")
                        nc.vector.tensor_tensor(out=o_sb[:], in0=r_sb[:],
                                                in1=psum_res[:], op=ALU.add)
                        nc.sync.dma_start(out=outT[:, tsl], in_=o_sb[:])

            for _ in range(repeat):
                body()

    nc.compile()
    return nc


def _assemble(results, meta):
    N = meta["N"]; shard = meta["shard"]; n_cores = meta["n_cores"]
    parts = []
    for c in range(n_cores):
        lo, hi = c * shard, min((c + 1) * shard, N)
        parts.append(results[c]["outT"][:, :hi - lo].T)
    return np.ascontiguousarray(np.concatenate(parts, axis=0).astype(np.float32))


def kernel(**inputs):
    in_maps, meta = _host_prep(**inputs)
    nc = _build(meta)
    res = run_bass_kernel_spmd(nc, in_maps, core_ids=list(range(N_CORES)))
    return _assemble(res.results, meta)

